# revision 25
# baseline (speedup 1.0000x reference)
"""FFJORD RK4 + Hutchinson trace kernel for 8x Trainium2 NeuronCores.

Strategy
--------
Pure data-parallel over the batch (65536 rows -> 8192 rows/core). Inside each
core, rows are processed in 8 "supertiles" of 1024 rows: two 512-row subtiles
(A, B) stacked on the 128 SBUF partitions (A feats on partitions 0-63, B on
64-127), features-major so the 3-layer MLP maps onto TensorE matmuls with the
batch as the moving (N=512) dimension.

The reference's finite-difference JVP is replaced by the analytic JVP
(identical for a piecewise-linear ReLU MLP up to rare kink crossings and the
reference's own fp32 cancellation noise ~1e-3):

    trace = e . (f(x + 0.5*eps_fd*e) - f(x)) / eps_fd  ~=  0.5 * e . (I + J_mlp) e

All matmuls run in float32r (TRN2 full-rate fp32 mode: inputs rounded to 11
mantissa bits, accumulation exact fp32), elementwise math in fp32. The scalar
time feature is folded into a per-step L1 bias table (b0 + t*W0[64]).
"""
import sys

sys.path.insert(0, "/opt/trn_rl_repo")

import numpy as np

import concourse.bass as bass
import concourse.tile as tile
from concourse import bacc, mybir
from concourse.bass_utils import run_bass_kernel_spmd

F32 = mybir.dt.float32
F32R = mybir.dt.float32r
AF = mybir.ActivationFunctionType
OP = mybir.AluOpType

NUM_STEPS = 16
FD_EPS = 1e-4
DT = 1.0 / NUM_STEPS
HALF_H = 0.5 * FD_EPS  # FD perturbation scale (folded into trace const)
D = 64
H = 256
N_CORES = 8
B_FULL = 65536
B_CORE = B_FULL // N_CORES  # 8192
BT = 512                    # batch columns per subtile
SUP_ROWS = 2 * BT           # rows per supertile (A|B stacked)
N_SUP = B_CORE // SUP_ROWS  # 8


def _round_f32r(x):
    """Round-to-nearest-even fp32 -> fp32r (11 explicit mantissa bits)."""
    u = np.ascontiguousarray(x, dtype=np.float32).view(np.uint32)
    lsb = (u >> 12) & 1
    u = (u + 0x7FF + lsb) & 0xFFFFF000
    return u.view(np.float32)


def _build(n_sup=N_SUP, n_steps=NUM_STEPS, b2_nonzero=False):
    nc = bacc.Bacc("TRN2", target_bir_lowering=False, debug=False,
                   enable_asserts=True, num_devices=N_CORES)
    rows = n_sup * SUP_ROWS

    x_d = nc.dram_tensor("x", [rows, D], F32, kind="ExternalInput").ap()
    e_d = nc.dram_tensor("eps", [n_steps, rows, D], F32, kind="ExternalInput").ap()
    w0_d = nc.dram_tensor("w0d", [128, H], F32R, kind="ExternalInput").ap()
    bt_d = nc.dram_tensor("b0t", [H, 2 * n_steps + 1], F32, kind="ExternalInput").ap()
    w1_d = nc.dram_tensor("w1", [H, H], F32R, kind="ExternalInput").ap()
    w2a_d = nc.dram_tensor("w2a", [H, 128], F32R, kind="ExternalInput").ap()
    w2b_d = nc.dram_tensor("w2b", [H, 128], F32R, kind="ExternalInput").ap()
    on_d = nc.dram_tensor("onesld", [128, 2], F32R, kind="ExternalInput").ap()
    id_d = nc.dram_tensor("ident", [128, 128], F32, kind="ExternalInput").ap()
    b2_d = nc.dram_tensor("b2d", [128, 1], F32, kind="ExternalInput").ap()
    xo_d = nc.dram_tensor("xo", [rows, D], F32, kind="ExternalOutput").ap()
    ld_d = nc.dram_tensor("ld", [rows], F32, kind="ExternalOutput").ap()

    with tile.TileContext(nc) as tc:
        with tc.tile_pool(name="wp", bufs=1) as wp, \
             tc.tile_pool(name="bm", bufs=3) as bmp, \
             tc.tile_pool(name="esb", bufs=3) as esp, \
             tc.tile_pool(name="hp", bufs=2) as hp, \
             tc.tile_pool(name="xs", bufs=2) as xsp, \
             tc.tile_pool(name="ldp", bufs=1) as ldp, \
             tc.tile_pool(name="wk", bufs=2) as wk, \
             tc.tile_pool(name="pp", bufs=2) as ppool, \
             tc.tile_pool(name="zp", bufs=3, space="PSUM") as zp, \
             tc.tile_pool(name="mp", bufs=1, space="PSUM") as mp, \
             tc.tile_pool(name="tpp", bufs=1, space="PSUM") as tpp:

            # ---- weights / constants (loaded once) ----
            w0 = wp.tile([128, H], F32R, tag="w0")
            nc.sync.dma_start(w0[:], w0_d[:])
            w1t = []
            for kg in range(2):
                row = []
                for mg in range(2):
                    t = wp.tile([128, 128], F32R, tag=f"w1_{kg}{mg}")
                    nc.sync.dma_start(
                        t[:], w1_d[kg * 128:(kg + 1) * 128, mg * 128:(mg + 1) * 128])
                    row.append(t)
                w1t.append(row)
            w2t = {}
            for half, wd in (("a", w2a_d), ("b", w2b_d)):
                for kg in range(2):
                    t = wp.tile([128, 128], F32R, tag=f"w2_{half}{kg}")
                    nc.sync.dma_start(t[:], wd[kg * 128:(kg + 1) * 128, :])
                    w2t[(half, kg)] = t
            b0t = []
            for mg in range(2):
                t = wp.tile([128, 2 * n_steps + 1], F32, tag=f"b0t_{mg}")
                nc.sync.dma_start(t[:], bt_d[mg * 128:(mg + 1) * 128, :])
                b0t.append(t)
            ones = wp.tile([128, 2], F32R, tag="ones")
            nc.sync.dma_start(ones[:], on_d[:])
            ident = wp.tile([128, 128], F32, tag="ident")
            nc.sync.dma_start(ident[:], id_d[:])
            b2t = wp.tile([128, 1], F32, tag="b2t")
            nc.sync.dma_start(b2t[:], b2_d[:])
            b2s = b2t[:, 0:1] if b2_nonzero else 0.0

            def transpose_in(dst_ps, src_bm):
                for j in range(4):
                    nc.tensor.transpose(
                        dst_ps[:, 128 * j:128 * j + 128],
                        src_bm[:, 128 * j:128 * j + 128],
                        ident[:, :])

            def stt(out, in0, scalar, in1, op0, op1):
                nc.vector.scalar_tensor_tensor(out, in0, scalar, in1, op0, op1)

            def mlp_g(xins, j, dve_l2):
                """Interleaved MLP evals for a group of supertiles.

                xins: list of [128, BT] f32r-rounded sbuf tiles.
                Returns (ms, h1s, h2s) lists, one entry per group member."""
                G = len(xins)
                zs1, h1s = [[None] * 2 for _ in range(G)], [[None] * 2 for _ in range(G)]
                for mg in range(2):
                    for g in range(G):
                        xr = xins[g][:].bitcast(F32R)
                        z = zp.tile([128, 2 * BT], F32, tag="z")
                        nc.tensor.matmul(z[:, 0:BT],
                                         w0[0:64, mg * 128:(mg + 1) * 128].bitcast(F32R),
                                         xr[0:64, :], start=True, stop=True)
                        nc.tensor.matmul(z[:, BT:2 * BT],
                                         w0[64:128, mg * 128:(mg + 1) * 128].bitcast(F32R),
                                         xr[64:128, :], start=True, stop=True)
                        zs1[g][mg] = z
                    for g in range(G):
                        h = hp.tile([128, 2 * BT], F32, tag=f"h1_{mg}")
                        nc.scalar.activation(h[:].bitcast(F32R), zs1[g][mg][:], AF.Relu,
                                             bias=b0t[mg][:, j:j + 1])
                        h1s[g][mg] = h
                zs2, h2s = [[None] * 2 for _ in range(G)], [[None] * 2 for _ in range(G)]
                for mg in range(2):
                    for g in range(G):
                        z = zp.tile([128, 2 * BT], F32, tag="z")
                        for kg in range(2):
                            st = (kg == 0)
                            sp = (kg == 1)
                            nc.tensor.matmul(z[:, 0:BT], w1t[kg][mg][:],
                                             h1s[g][kg][:, 0:BT].bitcast(F32R),
                                             start=st, stop=sp)
                            nc.tensor.matmul(z[:, BT:2 * BT], w1t[kg][mg][:],
                                             h1s[g][kg][:, BT:2 * BT].bitcast(F32R),
                                             start=st, stop=sp)
                        zs2[g][mg] = z
                    for g in range(G):
                        h = hp.tile([128, 2 * BT], F32, tag=f"h2_{mg}")
                        if dve_l2 and mg == 1:
                            nc.vector.tensor_scalar_max(h[:].bitcast(F32R),
                                                        zs2[g][mg][:], 0.0)
                        else:
                            nc.scalar.activation(h[:].bitcast(F32R), zs2[g][mg][:],
                                                 AF.Relu)
                        h2s[g][mg] = h
                ms = []
                for g in range(G):
                    m = mp.tile([128, BT], F32, tag="m")
                    first = True
                    for half in ("a", "b"):
                        lo, hi = (0, BT) if half == "a" else (BT, 2 * BT)
                        for kg in range(2):
                            nc.tensor.matmul(m[:], w2t[(half, kg)][:],
                                             h2s[g][kg][:, lo:hi].bitcast(F32R),
                                             start=first,
                                             stop=(half == "b" and kg == 1))
                            first = False
                    ms.append(m)
                return ms, h1s, h2s

            def tangent_g(esbs, h1s, h2s):
                G = len(esbs)
                v1s, v2s = [[None] * 2 for _ in range(G)], [[None] * 2 for _ in range(G)]
                for mg in range(2):
                    us = [None] * G
                    for g in range(G):
                        er = esbs[g][:].bitcast(F32R)
                        u = zp.tile([128, 2 * BT], F32, tag="z")
                        nc.tensor.matmul(u[:, 0:BT],
                                         w0[0:64, mg * 128:(mg + 1) * 128].bitcast(F32R),
                                         er[0:64, :], start=True, stop=True)
                        nc.tensor.matmul(u[:, BT:2 * BT],
                                         w0[64:128, mg * 128:(mg + 1) * 128].bitcast(F32R),
                                         er[64:128, :], start=True, stop=True)
                        us[g] = u
                    for g in range(G):
                        v = hp.tile([128, 2 * BT], F32, tag=f"v1_{mg}")
                        stt(v[:].bitcast(F32R), h1s[g][mg][:], 0.0, us[g][:],
                            OP.is_gt, OP.mult)
                        v1s[g][mg] = v
                for mg in range(2):
                    us = [None] * G
                    for g in range(G):
                        u = zp.tile([128, 2 * BT], F32, tag="z")
                        for kg in range(2):
                            st = (kg == 0)
                            sp = (kg == 1)
                            nc.tensor.matmul(u[:, 0:BT], w1t[kg][mg][:],
                                             v1s[g][kg][:, 0:BT].bitcast(F32R),
                                             start=st, stop=sp)
                            nc.tensor.matmul(u[:, BT:2 * BT], w1t[kg][mg][:],
                                             v1s[g][kg][:, BT:2 * BT].bitcast(F32R),
                                             start=st, stop=sp)
                        us[g] = u
                    for g in range(G):
                        v = hp.tile([128, 2 * BT], F32, tag=f"v2_{mg}")
                        stt(v[:].bitcast(F32R), h2s[g][mg][:], 0.0, us[g][:],
                            OP.is_gt, OP.mult)
                        v2s[g][mg] = v
                jvs = []
                for g in range(G):
                    jv = mp.tile([128, BT], F32, tag="m")
                    first = True
                    for half in ("a", "b"):
                        lo, hi = (0, BT) if half == "a" else (BT, 2 * BT)
                        for kg in range(2):
                            nc.tensor.matmul(jv[:], w2t[(half, kg)][:],
                                             v2s[g][kg][:, lo:hi].bitcast(F32R),
                                             start=first,
                                             stop=(half == "b" and kg == 1))
                            first = False
                    jvs.append(jv)
                return jvs

            GROUP = 2
            n_grp = (n_sup + GROUP - 1) // GROUP
            groups = [list(range(g * GROUP, min((g + 1) * GROUP, n_sup)))
                      for g in range(n_grp)]

            # ---- pass 1: load + transpose x ----
            xcs = [None] * n_sup
            ldas = [None] * n_sup
            for sup in range(n_sup):
                r0 = sup * SUP_ROWS
                x_view = x_d[r0:r0 + SUP_ROWS, :].rearrange(
                    "(q p) d -> p q d", q=8)
                xbm = bmp.tile([128, 512], F32, tag="xbm")
                nc.sync.dma_start(
                    xbm[:].rearrange("p (q d) -> p q d", q=8), x_view)
                pst = tpp.tile([128, 512], F32, tag="pst")
                transpose_in(pst, xbm)
                xc = xsp.tile([128, BT], F32, tag=f"xst{sup}")
                nc.scalar.activation(xc[:].bitcast(F32R), pst[:], AF.Copy)
                xcs[sup] = xc
                lda = ldp.tile([128, BT], F32, tag=f"lda{sup}")
                ldas[sup] = lda

            # ---- pass 2: steps outermost, groups of supertiles interleaved ----
            for s in range(n_steps):
                j0 = 2 * s
                for sups in groups:
                    G = len(sups)
                    # eps load + transpose + round
                    esbs = []
                    for sup in sups:
                        r0 = sup * SUP_ROWS
                        ebm = bmp.tile([128, 512], F32, tag="ebm")
                        nc.sync.dma_start(
                            ebm[:].rearrange("p (q d) -> p q d", q=8),
                            e_d[s, r0:r0 + SUP_ROWS, :].rearrange(
                                "(q p) d -> p q d", q=8))
                        pse = tpp.tile([128, 512], F32, tag="pst")
                        transpose_in(pse, ebm)
                        e_sb = esp.tile([128, BT], F32, tag="esb")
                        nc.scalar.activation(e_sb[:].bitcast(F32R), pse[:], AF.Copy)
                        esbs.append(e_sb)

                    xg = [xcs[sup] for sup in sups]
                    m1s, h1s, h2s = mlp_g(xg, j0, dve_l2=False)
                    k1s = []
                    for g in range(G):
                        k1 = wk.tile([128, BT], F32, tag="k1")
                        stt(k1[:], m1s[g][:], b2s, xg[g][:], OP.add, OP.add)
                        k1s.append(k1)

                    jvs = tangent_g(esbs, h1s, h2s)
                    for g in range(G):
                        st_ = wk.tile([128, BT], F32, tag="st")
                        nc.vector.tensor_tensor(st_[:], esbs[g][:], jvs[g][:],
                                                op=OP.add)
                        lda = ldas[sups[g]]
                        if s == 0:
                            nc.vector.tensor_tensor(lda[:], esbs[g][:], st_[:],
                                                    op=OP.mult)
                        else:
                            p = ppool.tile([128, BT], F32, tag="p")
                            nc.vector.tensor_tensor(p[:], esbs[g][:], st_[:],
                                                    op=OP.mult)
                            nc.vector.tensor_tensor(lda[:], lda[:], p[:], op=OP.add)

                    x2s = []
                    for g in range(G):
                        x2 = wk.tile([128, BT], F32, tag="x2")
                        stt(x2[:].bitcast(F32R), k1s[g][:], DT / 2, xg[g][:],
                            OP.mult, OP.add)
                        x2s.append(x2)
                    m2s, _, _ = mlp_g(x2s, j0 + 1, dve_l2=True)
                    k2s = []
                    for g in range(G):
                        k2 = wk.tile([128, BT], F32, tag="k2")
                        stt(k2[:], m2s[g][:], b2s, x2s[g][:], OP.add, OP.add)
                        k2s.append(k2)

                    x3s = []
                    for g in range(G):
                        x3 = wk.tile([128, BT], F32, tag="x3")
                        stt(x3[:].bitcast(F32R), k2s[g][:], DT / 2, xg[g][:],
                            OP.mult, OP.add)
                        x3s.append(x3)
                    m3s, _, _ = mlp_g(x3s, j0 + 1, dve_l2=False)
                    k3s = []
                    for g in range(G):
                        k3 = wk.tile([128, BT], F32, tag="k3")
                        stt(k3[:], m3s[g][:], b2s, x3s[g][:], OP.add, OP.add)
                        k3s.append(k3)

                    x4s = []
                    for g in range(G):
                        x4 = wk.tile([128, BT], F32, tag="x4")
                        stt(x4[:].bitcast(F32R), k3s[g][:], DT, xg[g][:],
                            OP.mult, OP.add)
                        x4s.append(x4)
                    m4s, _, _ = mlp_g(x4s, j0 + 2, dve_l2=False)
                    for g in range(G):
                        k4 = wk.tile([128, BT], F32, tag="k4")
                        stt(k4[:], m4s[g][:], b2s, x4s[g][:], OP.add, OP.add)
                        u = wk.tile([128, BT], F32, tag="u")
                        nc.vector.tensor_tensor(u[:], k1s[g][:], k4[:], op=OP.add)
                        v = wk.tile([128, BT], F32, tag="v")
                        nc.vector.tensor_tensor(v[:], k2s[g][:], k3s[g][:], op=OP.add)
                        w = wk.tile([128, BT], F32, tag="w")
                        stt(w[:], v[:], 2.0, u[:], OP.mult, OP.add)
                        xn = xsp.tile([128, BT], F32, tag=f"xst{sups[g]}")
                        stt(xn[:].bitcast(F32R), w[:], DT / 6, xg[g][:],
                            OP.mult, OP.add)
                        xcs[sups[g]] = xn

            # ---- pass 3: outputs ----
            for sup in range(n_sup):
                r0 = sup * SUP_ROWS
                xo_view = xo_d[r0:r0 + SUP_ROWS, :].rearrange(
                    "(q p) d -> p q d", q=8)
                xc = xcs[sup]
                ld_acc = ldas[sup]
                pso = tpp.tile([128, 512], F32, tag="pst")
                for j in range(4):
                    nc.tensor.transpose(
                        pso[:, 128 * j:128 * j + 128],
                        xc[:, 128 * j:128 * j + 128],
                        ident[:, :])
                xob = bmp.tile([128, 512], F32, tag="xbm")
                nc.scalar.activation(xob[:], pso[:], AF.Copy)
                nc.sync.dma_start(
                    xo_view, xob[:].rearrange("p (q d) -> p q d", q=8))

                ldr = ppool.tile([128, BT], F32, tag="p")
                nc.scalar.activation(ldr[:].bitcast(F32R), ld_acc[:], AF.Copy)
                trace_ps = mp.tile([2, BT], F32, tag="m")
                nc.tensor.matmul(trace_ps[:], ones[:], ldr[:].bitcast(F32R),
                                 start=True, stop=True)
                ld_view = ld_d[r0:r0 + SUP_ROWS].rearrange(
                    "(g sub k) -> sub g k", g=4, sub=2)
                ldb = ppool.tile([2, BT], F32, tag="ldb")
                nc.scalar.activation(ldb[:], trace_ps[:], AF.Copy)
                nc.sync.dma_start(ld_view, ldb[:])

    nc.compile()
    return nc


_CACHE = {}


def _get_executor(b2_nonzero):
    """Build (once) and cache a jitted 8-core shard_map executor.

    Returns (run, in_names, out_names, out_shapes) where
    run(concat_inputs: list[np.ndarray]) -> list of per-output global arrays.
    """
    key = b2_nonzero
    if key in _CACHE:
        return _CACHE[key]
    import jax
    from jax.sharding import Mesh, PartitionSpec
    from jax.experimental.shard_map import shard_map
    from concourse import bass2jax
    from concourse.bass2jax import (_bass_exec_p, install_neuronx_cc_hook,
                                    partition_id_tensor)

    nc = _build(b2_nonzero=b2_nonzero)
    install_neuronx_cc_hook()

    part_name = nc.partition_id_tensor.name if nc.partition_id_tensor else None
    in_names, out_names, out_avals = [], [], []
    for alloc in nc.m.functions[0].allocations:
        if not isinstance(alloc, mybir.MemoryLocationSet):
            continue
        name = alloc.memorylocations[0].name
        if alloc.kind == "ExternalInput":
            if name != part_name:
                in_names.append(name)
        elif alloc.kind == "ExternalOutput":
            out_names.append(name)
            out_avals.append(jax.core.ShapedArray(
                tuple(alloc.tensor_shape), mybir.dt.np(alloc.dtype)))
    n_params = len(in_names)
    n_outs = len(out_names)
    all_in_names = in_names + out_names
    if part_name is not None:
        all_in_names = all_in_names + [part_name]

    def _body(*args):
        operands = list(args)
        if part_name is not None:
            operands.append(partition_id_tensor())
        outs = _bass_exec_p.bind(
            *operands,
            out_avals=tuple(out_avals),
            in_names=tuple(all_in_names),
            out_names=tuple(out_names),
            lowering_input_output_aliases=(),
            sim_require_finite=True,
            sim_require_nnan=True,
            nc=nc,
        )
        return tuple(outs)

    devices = jax.devices()[:N_CORES]
    mesh = Mesh(np.asarray(devices), ("core",))
    donate = tuple(range(n_params, n_params + n_outs))
    sharded = jax.jit(
        shard_map(_body, mesh=mesh,
                  in_specs=(PartitionSpec("core"),) * (n_params + n_outs),
                  out_specs=(PartitionSpec("core"),) * n_outs,
                  check_rep=False),
        donate_argnums=donate, keep_unused=True)

    def run(concat_inputs):
        zeros = [np.zeros((N_CORES * a.shape[0], *a.shape[1:]), a.dtype)
                 for a in out_avals]
        outs = sharded(*concat_inputs, *zeros)
        return [np.asarray(o) for o in outs]

    out_shapes = [tuple(a.shape) for a in out_avals]
    _CACHE[key] = (run, in_names, out_names, out_shapes)
    _CACHE[(key, "bench")] = (sharded, out_avals)
    return _CACHE[key]


def kernel(x, eps, W0, b0, W1, b1, W2, b2):
    x = np.asarray(x, np.float32)
    eps = np.asarray(eps, np.float32)
    W0 = np.asarray(W0, np.float32)
    b0 = np.asarray(b0, np.float32)
    W1 = np.asarray(W1, np.float32)
    b1 = np.asarray(b1, np.float32)
    W2 = np.asarray(W2, np.float32)
    b2 = np.asarray(b2, np.float32)

    b2_nonzero = bool(np.any(b2))
    run, in_names, out_names, _ = _get_executor(b2_nonzero)

    # host-side weight prep
    w0d = _round_f32r(np.concatenate([W0[:D], W0[:D]], axis=0))      # [128, 256]
    # L1 bias table: col j = b0 + t_j * W0[64], t_j = j*dt/2, plus b1 folded? no.
    tgrid = (np.arange(2 * NUM_STEPS + 1, dtype=np.float64) * (DT / 2))
    b0t = (b0[None, :].astype(np.float64)
           + tgrid[:, None] * W0[D].astype(np.float64)).astype(np.float32)  # [33, H]
    b0t = np.ascontiguousarray(b0t.T)                                 # [H, 33]
    # NOTE: b1 folding - L2 bias is b1 (zeros in spec); if nonzero we add via
    # the relu bias path. Handled below by asserting zero for the fast path.
    if np.any(b1):
        raise NotImplementedError("nonzero b1 not supported by this kernel build")
    w1r = _round_f32r(W1)
    w2a = np.zeros((H, 128), np.float32)
    w2a[:, :D] = W2
    w2b = np.zeros((H, 128), np.float32)
    w2b[:, D:] = W2
    w2a = _round_f32r(w2a)
    w2b = _round_f32r(w2b)
    onesld = np.zeros((128, 2), np.float32)
    c_star = -DT * 0.5  # trace = 0.5*e.(e+Jmlp e); ld -= dt*trace
    onesld[0:64, 0] = c_star
    onesld[64:128, 1] = c_star
    onesld = _round_f32r(onesld)
    ident = np.eye(128, dtype=np.float32)
    b2d = np.concatenate([b2, b2]).reshape(128, 1).astype(np.float32)

    # Global (concatenated over cores) input arrays for shard_map:
    # per-core eps shard is eps[:, r0:r0+B_CORE, :]; concatenated along axis 0
    # that is eps transposed to [cores*steps, B_CORE, D].
    eps_g = np.ascontiguousarray(
        eps.reshape(NUM_STEPS, N_CORES, B_CORE, D).transpose(1, 0, 2, 3)
    ).reshape(N_CORES * NUM_STEPS, B_CORE, D)
    per_name = {
        "x": x,                               # [8*8192, 64] already global
        "eps": eps_g,
        "w0d": np.tile(w0d, (N_CORES, 1)),
        "b0t": np.tile(b0t, (N_CORES, 1)),
        "w1": np.tile(w1r, (N_CORES, 1)),
        "w2a": np.tile(w2a, (N_CORES, 1)),
        "w2b": np.tile(w2b, (N_CORES, 1)),
        "onesld": np.tile(onesld, (N_CORES, 1)),
        "ident": np.tile(ident, (N_CORES, 1)),
        "b2d": np.tile(b2d, (N_CORES, 1)),
    }
    outs = run([per_name[n] for n in in_names])
    res = dict(zip(out_names, outs))
    x_out = res["xo"]
    log_det = res["ld"]
    return x_out, log_det


# revision 27
# speedup vs baseline: 1.0804x; 1.0804x over previous
"""FFJORD RK4 + Hutchinson trace kernel for 8x Trainium2 NeuronCores.

Strategy
--------
Pure data-parallel over the batch (65536 rows -> 8192 rows/core). Inside each
core, rows are processed in 8 "supertiles" of 1024 rows: two 512-row subtiles
(A, B) stacked on the 128 SBUF partitions (A feats on partitions 0-63, B on
64-127), features-major so the 3-layer MLP maps onto TensorE matmuls with the
batch as the moving (N=512) dimension.

The reference's finite-difference JVP is replaced by the analytic JVP
(identical for a piecewise-linear ReLU MLP up to rare kink crossings and the
reference's own fp32 cancellation noise ~1e-3):

    trace = e . (f(x + 0.5*eps_fd*e) - f(x)) / eps_fd  ~=  0.5 * e . (I + J_mlp) e

All matmuls run in float32r (TRN2 full-rate fp32 mode: inputs rounded to 11
mantissa bits, accumulation exact fp32), elementwise math in fp32. The scalar
time feature is folded into a per-step L1 bias table (b0 + t*W0[64]).
"""
import sys

sys.path.insert(0, "/opt/trn_rl_repo")

import numpy as np

import concourse.bass as bass
import concourse.tile as tile
from concourse import bacc, mybir
from concourse.bass_utils import run_bass_kernel_spmd

F32 = mybir.dt.float32
F32R = mybir.dt.float32r
AF = mybir.ActivationFunctionType
OP = mybir.AluOpType

NUM_STEPS = 16
FD_EPS = 1e-4
DT = 1.0 / NUM_STEPS
HALF_H = 0.5 * FD_EPS  # FD perturbation scale (folded into trace const)
D = 64
H = 256
N_CORES = 8
B_FULL = 65536
B_CORE = B_FULL // N_CORES  # 8192
BT = 512                    # batch columns per subtile
SUP_ROWS = 2 * BT           # rows per supertile (A|B stacked)
N_SUP = B_CORE // SUP_ROWS  # 8


def _round_f32r(x):
    """Round-to-nearest-even fp32 -> fp32r (11 explicit mantissa bits)."""
    u = np.ascontiguousarray(x, dtype=np.float32).view(np.uint32)
    lsb = (u >> 12) & 1
    u = (u + 0x7FF + lsb) & 0xFFFFF000
    return u.view(np.float32)


def _build(n_sup=N_SUP, n_steps=NUM_STEPS, b2_nonzero=False, b1_nonzero=False):
    nc = bacc.Bacc("TRN2", target_bir_lowering=False, debug=False,
                   enable_asserts=True, num_devices=N_CORES)
    rows = n_sup * SUP_ROWS

    x_d = nc.dram_tensor("x", [rows, D], F32, kind="ExternalInput").ap()
    e_d = nc.dram_tensor("eps", [n_steps, rows, D], F32, kind="ExternalInput").ap()
    w0_d = nc.dram_tensor("w0d", [128, H], F32R, kind="ExternalInput").ap()
    bt_d = nc.dram_tensor("b0t", [H, 2 * n_steps + 1], F32, kind="ExternalInput").ap()
    w1_d = nc.dram_tensor("w1", [H, H], F32R, kind="ExternalInput").ap()
    w2a_d = nc.dram_tensor("w2a", [H, 128], F32R, kind="ExternalInput").ap()
    w2b_d = nc.dram_tensor("w2b", [H, 128], F32R, kind="ExternalInput").ap()
    on_d = nc.dram_tensor("onesld", [128, 2], F32R, kind="ExternalInput").ap()
    id_d = nc.dram_tensor("ident", [128, 128], F32, kind="ExternalInput").ap()
    b2_d = nc.dram_tensor("b2d", [128, 1], F32, kind="ExternalInput").ap()
    b1_d = nc.dram_tensor("b1d", [H, 1], F32, kind="ExternalInput").ap()
    xo_d = nc.dram_tensor("xo", [rows, D], F32, kind="ExternalOutput").ap()
    ld_d = nc.dram_tensor("ld", [rows], F32, kind="ExternalOutput").ap()

    with tile.TileContext(nc) as tc:
        with tc.tile_pool(name="wp", bufs=1) as wp, \
             tc.tile_pool(name="bm", bufs=3) as bmp, \
             tc.tile_pool(name="esb", bufs=3) as esp, \
             tc.tile_pool(name="hp", bufs=2) as hp, \
             tc.tile_pool(name="xs", bufs=2) as xsp, \
             tc.tile_pool(name="ldp", bufs=1) as ldp, \
             tc.tile_pool(name="wk", bufs=2) as wk, \
             tc.tile_pool(name="pp", bufs=2) as ppool, \
             tc.tile_pool(name="zp", bufs=3, space="PSUM") as zp, \
             tc.tile_pool(name="mp", bufs=1, space="PSUM") as mp, \
             tc.tile_pool(name="tpp", bufs=1, space="PSUM") as tpp:

            # ---- weights / constants (loaded once) ----
            w0 = wp.tile([128, H], F32R, tag="w0")
            nc.sync.dma_start(w0[:], w0_d[:])
            w1t = []
            for kg in range(2):
                row = []
                for mg in range(2):
                    t = wp.tile([128, 128], F32R, tag=f"w1_{kg}{mg}")
                    nc.sync.dma_start(
                        t[:], w1_d[kg * 128:(kg + 1) * 128, mg * 128:(mg + 1) * 128])
                    row.append(t)
                w1t.append(row)
            w2t = {}
            for half, wd in (("a", w2a_d), ("b", w2b_d)):
                for kg in range(2):
                    t = wp.tile([128, 128], F32R, tag=f"w2_{half}{kg}")
                    nc.sync.dma_start(t[:], wd[kg * 128:(kg + 1) * 128, :])
                    w2t[(half, kg)] = t
            b0t = []
            for mg in range(2):
                t = wp.tile([128, 2 * n_steps + 1], F32, tag=f"b0t_{mg}")
                nc.sync.dma_start(t[:], bt_d[mg * 128:(mg + 1) * 128, :])
                b0t.append(t)
            ones = wp.tile([128, 2], F32R, tag="ones")
            nc.sync.dma_start(ones[:], on_d[:])
            ident = wp.tile([128, 128], F32, tag="ident")
            nc.sync.dma_start(ident[:], id_d[:])
            b2t = wp.tile([128, 1], F32, tag="b2t")
            nc.sync.dma_start(b2t[:], b2_d[:])
            b2s = b2t[:, 0:1] if b2_nonzero else 0.0
            b1t = []
            for mg in range(2):
                t = wp.tile([128, 1], F32, tag=f"b1t_{mg}")
                nc.sync.dma_start(t[:], b1_d[mg * 128:(mg + 1) * 128, :])
                b1t.append(t)

            def transpose_in(dst_ps, src_bm):
                for j in range(4):
                    nc.tensor.transpose(
                        dst_ps[:, 128 * j:128 * j + 128],
                        src_bm[:, 128 * j:128 * j + 128],
                        ident[:, :])

            def stt(out, in0, scalar, in1, op0, op1):
                nc.vector.scalar_tensor_tensor(out, in0, scalar, in1, op0, op1)

            def mlp_g(xins, j, dve_l2):
                """Interleaved MLP evals for a group of supertiles.

                xins: list of [128, BT] f32r-rounded sbuf tiles.
                Returns (ms, h1s, h2s) lists, one entry per group member."""
                G = len(xins)
                zs1, h1s = [[None] * 2 for _ in range(G)], [[None] * 2 for _ in range(G)]
                for mg in range(2):
                    for g in range(G):
                        xr = xins[g][:].bitcast(F32R)
                        z = zp.tile([128, 2 * BT], F32, tag="z")
                        nc.tensor.matmul(z[:, 0:BT],
                                         w0[0:64, mg * 128:(mg + 1) * 128].bitcast(F32R),
                                         xr[0:64, :], start=True, stop=True)
                        nc.tensor.matmul(z[:, BT:2 * BT],
                                         w0[64:128, mg * 128:(mg + 1) * 128].bitcast(F32R),
                                         xr[64:128, :], start=True, stop=True)
                        zs1[g][mg] = z
                    for g in range(G):
                        h = hp.tile([128, 2 * BT], F32, tag=f"h1_{mg}")
                        nc.scalar.activation(h[:].bitcast(F32R), zs1[g][mg][:], AF.Relu,
                                             bias=b0t[mg][:, j:j + 1])
                        h1s[g][mg] = h
                zs2, h2s = [[None] * 2 for _ in range(G)], [[None] * 2 for _ in range(G)]
                for mg in range(2):
                    for g in range(G):
                        z = zp.tile([128, 2 * BT], F32, tag="z")
                        for kg in range(2):
                            st = (kg == 0)
                            sp = (kg == 1)
                            nc.tensor.matmul(z[:, 0:BT], w1t[kg][mg][:],
                                             h1s[g][kg][:, 0:BT].bitcast(F32R),
                                             start=st, stop=sp)
                            nc.tensor.matmul(z[:, BT:2 * BT], w1t[kg][mg][:],
                                             h1s[g][kg][:, BT:2 * BT].bitcast(F32R),
                                             start=st, stop=sp)
                        zs2[g][mg] = z
                    for g in range(G):
                        h = hp.tile([128, 2 * BT], F32, tag=f"h2_{mg}")
                        if dve_l2 and mg == 1:
                            if b1_nonzero:
                                nc.vector.tensor_scalar(
                                    h[:].bitcast(F32R), zs2[g][mg][:],
                                    b1t[mg][:, 0:1], 0.0, OP.add, OP.max)
                            else:
                                nc.vector.tensor_scalar_max(h[:].bitcast(F32R),
                                                            zs2[g][mg][:], 0.0)
                        else:
                            if b1_nonzero:
                                nc.scalar.activation(h[:].bitcast(F32R),
                                                     zs2[g][mg][:], AF.Relu,
                                                     bias=b1t[mg][:, 0:1])
                            else:
                                nc.scalar.activation(h[:].bitcast(F32R),
                                                     zs2[g][mg][:], AF.Relu)
                        h2s[g][mg] = h
                ms = []
                for g in range(G):
                    m = mp.tile([128, BT], F32, tag="m")
                    first = True
                    for half in ("a", "b"):
                        lo, hi = (0, BT) if half == "a" else (BT, 2 * BT)
                        for kg in range(2):
                            nc.tensor.matmul(m[:], w2t[(half, kg)][:],
                                             h2s[g][kg][:, lo:hi].bitcast(F32R),
                                             start=first,
                                             stop=(half == "b" and kg == 1))
                            first = False
                    ms.append(m)
                return ms, h1s, h2s

            def tangent_g(esbs, h1s, h2s):
                G = len(esbs)
                v1s, v2s = [[None] * 2 for _ in range(G)], [[None] * 2 for _ in range(G)]
                for mg in range(2):
                    us = [None] * G
                    for g in range(G):
                        er = esbs[g][:].bitcast(F32R)
                        u = zp.tile([128, 2 * BT], F32, tag="z")
                        nc.tensor.matmul(u[:, 0:BT],
                                         w0[0:64, mg * 128:(mg + 1) * 128].bitcast(F32R),
                                         er[0:64, :], start=True, stop=True)
                        nc.tensor.matmul(u[:, BT:2 * BT],
                                         w0[64:128, mg * 128:(mg + 1) * 128].bitcast(F32R),
                                         er[64:128, :], start=True, stop=True)
                        us[g] = u
                    for g in range(G):
                        v = hp.tile([128, 2 * BT], F32, tag=f"v1_{mg}")
                        stt(v[:].bitcast(F32R), h1s[g][mg][:], 0.0, us[g][:],
                            OP.is_gt, OP.mult)
                        v1s[g][mg] = v
                for mg in range(2):
                    us = [None] * G
                    for g in range(G):
                        u = zp.tile([128, 2 * BT], F32, tag="z")
                        for kg in range(2):
                            st = (kg == 0)
                            sp = (kg == 1)
                            nc.tensor.matmul(u[:, 0:BT], w1t[kg][mg][:],
                                             v1s[g][kg][:, 0:BT].bitcast(F32R),
                                             start=st, stop=sp)
                            nc.tensor.matmul(u[:, BT:2 * BT], w1t[kg][mg][:],
                                             v1s[g][kg][:, BT:2 * BT].bitcast(F32R),
                                             start=st, stop=sp)
                        us[g] = u
                    for g in range(G):
                        v = hp.tile([128, 2 * BT], F32, tag=f"v2_{mg}")
                        stt(v[:].bitcast(F32R), h2s[g][mg][:], 0.0, us[g][:],
                            OP.is_gt, OP.mult)
                        v2s[g][mg] = v
                jvs = []
                for g in range(G):
                    jv = mp.tile([128, BT], F32, tag="m")
                    first = True
                    for half in ("a", "b"):
                        lo, hi = (0, BT) if half == "a" else (BT, 2 * BT)
                        for kg in range(2):
                            nc.tensor.matmul(jv[:], w2t[(half, kg)][:],
                                             v2s[g][kg][:, lo:hi].bitcast(F32R),
                                             start=first,
                                             stop=(half == "b" and kg == 1))
                            first = False
                    jvs.append(jv)
                return jvs

            GROUP = 2
            n_grp = (n_sup + GROUP - 1) // GROUP
            groups = [list(range(g * GROUP, min((g + 1) * GROUP, n_sup)))
                      for g in range(n_grp)]

            # ---- pass 1: load + transpose x ----
            xcs = [None] * n_sup
            ldas = [None] * n_sup
            for sup in range(n_sup):
                r0 = sup * SUP_ROWS
                x_view = x_d[r0:r0 + SUP_ROWS, :].rearrange(
                    "(q p) d -> p q d", q=8)
                xbm = bmp.tile([128, 512], F32, tag="xbm")
                nc.sync.dma_start(
                    xbm[:].rearrange("p (q d) -> p q d", q=8), x_view)
                pst = tpp.tile([128, 512], F32, tag="pst")
                transpose_in(pst, xbm)
                xc = xsp.tile([128, BT], F32, tag=f"xst{sup}")
                nc.scalar.activation(xc[:].bitcast(F32R), pst[:], AF.Copy)
                xcs[sup] = xc
                lda = ldp.tile([128, BT], F32, tag=f"lda{sup}")
                ldas[sup] = lda

            # ---- pass 2: steps outermost, groups of supertiles interleaved ----
            for s in range(n_steps):
                j0 = 2 * s
                for sups in groups:
                    G = len(sups)
                    # eps load + transpose + round
                    esbs = []
                    for sup in sups:
                        r0 = sup * SUP_ROWS
                        ebm = bmp.tile([128, 512], F32, tag="ebm")
                        nc.sync.dma_start(
                            ebm[:].rearrange("p (q d) -> p q d", q=8),
                            e_d[s, r0:r0 + SUP_ROWS, :].rearrange(
                                "(q p) d -> p q d", q=8))
                        pse = tpp.tile([128, 512], F32, tag="pst")
                        transpose_in(pse, ebm)
                        e_sb = esp.tile([128, BT], F32, tag="esb")
                        nc.scalar.activation(e_sb[:].bitcast(F32R), pse[:], AF.Copy)
                        esbs.append(e_sb)

                    xg = [xcs[sup] for sup in sups]
                    m1s, h1s, h2s = mlp_g(xg, j0, dve_l2=False)
                    k1s = []
                    for g in range(G):
                        k1 = wk.tile([128, BT], F32, tag="k1")
                        stt(k1[:], m1s[g][:], b2s, xg[g][:], OP.add, OP.add)
                        k1s.append(k1)

                    jvs = tangent_g(esbs, h1s, h2s)
                    for g in range(G):
                        st_ = wk.tile([128, BT], F32, tag="st")
                        nc.vector.tensor_tensor(st_[:], esbs[g][:], jvs[g][:],
                                                op=OP.add)
                        lda = ldas[sups[g]]
                        if s == 0:
                            nc.vector.tensor_tensor(lda[:], esbs[g][:], st_[:],
                                                    op=OP.mult)
                        else:
                            p = ppool.tile([128, BT], F32, tag="p")
                            nc.vector.tensor_tensor(p[:], esbs[g][:], st_[:],
                                                    op=OP.mult)
                            nc.vector.tensor_tensor(lda[:], lda[:], p[:], op=OP.add)

                    x2s = []
                    for g in range(G):
                        x2 = wk.tile([128, BT], F32, tag="x2")
                        stt(x2[:].bitcast(F32R), k1s[g][:], DT / 2, xg[g][:],
                            OP.mult, OP.add)
                        x2s.append(x2)
                    m2s, _, _ = mlp_g(x2s, j0 + 1, dve_l2=True)
                    k2s = []
                    for g in range(G):
                        k2 = wk.tile([128, BT], F32, tag="k2")
                        stt(k2[:], m2s[g][:], b2s, x2s[g][:], OP.add, OP.add)
                        k2s.append(k2)

                    x3s = []
                    for g in range(G):
                        x3 = wk.tile([128, BT], F32, tag="x3")
                        stt(x3[:].bitcast(F32R), k2s[g][:], DT / 2, xg[g][:],
                            OP.mult, OP.add)
                        x3s.append(x3)
                    m3s, _, _ = mlp_g(x3s, j0 + 1, dve_l2=False)
                    k3s = []
                    for g in range(G):
                        k3 = wk.tile([128, BT], F32, tag="k3")
                        stt(k3[:], m3s[g][:], b2s, x3s[g][:], OP.add, OP.add)
                        k3s.append(k3)

                    x4s = []
                    for g in range(G):
                        x4 = wk.tile([128, BT], F32, tag="x4")
                        stt(x4[:].bitcast(F32R), k3s[g][:], DT, xg[g][:],
                            OP.mult, OP.add)
                        x4s.append(x4)
                    m4s, _, _ = mlp_g(x4s, j0 + 2, dve_l2=False)
                    for g in range(G):
                        k4 = wk.tile([128, BT], F32, tag="k4")
                        stt(k4[:], m4s[g][:], b2s, x4s[g][:], OP.add, OP.add)
                        u = wk.tile([128, BT], F32, tag="u")
                        nc.vector.tensor_tensor(u[:], k1s[g][:], k4[:], op=OP.add)
                        v = wk.tile([128, BT], F32, tag="v")
                        nc.vector.tensor_tensor(v[:], k2s[g][:], k3s[g][:], op=OP.add)
                        w = wk.tile([128, BT], F32, tag="w")
                        stt(w[:], v[:], 2.0, u[:], OP.mult, OP.add)
                        xn = xsp.tile([128, BT], F32, tag=f"xst{sups[g]}")
                        stt(xn[:].bitcast(F32R), w[:], DT / 6, xg[g][:],
                            OP.mult, OP.add)
                        xcs[sups[g]] = xn

            # ---- pass 3: outputs ----
            for sup in range(n_sup):
                r0 = sup * SUP_ROWS
                xo_view = xo_d[r0:r0 + SUP_ROWS, :].rearrange(
                    "(q p) d -> p q d", q=8)
                xc = xcs[sup]
                ld_acc = ldas[sup]
                pso = tpp.tile([128, 512], F32, tag="pst")
                for j in range(4):
                    nc.tensor.transpose(
                        pso[:, 128 * j:128 * j + 128],
                        xc[:, 128 * j:128 * j + 128],
                        ident[:, :])
                xob = bmp.tile([128, 512], F32, tag="xbm")
                nc.scalar.activation(xob[:], pso[:], AF.Copy)
                nc.sync.dma_start(
                    xo_view, xob[:].rearrange("p (q d) -> p q d", q=8))

                ldr = ppool.tile([128, BT], F32, tag="p")
                nc.scalar.activation(ldr[:].bitcast(F32R), ld_acc[:], AF.Copy)
                trace_ps = mp.tile([2, BT], F32, tag="m")
                nc.tensor.matmul(trace_ps[:], ones[:], ldr[:].bitcast(F32R),
                                 start=True, stop=True)
                ld_view = ld_d[r0:r0 + SUP_ROWS].rearrange(
                    "(g sub k) -> sub g k", g=4, sub=2)
                ldb = ppool.tile([2, BT], F32, tag="ldb")
                nc.scalar.activation(ldb[:], trace_ps[:], AF.Copy)
                nc.sync.dma_start(ld_view, ldb[:])

    nc.compile()
    return nc


_CACHE = {}


def _get_executor(b2_nonzero, b1_nonzero):
    """Build (once) and cache a jitted 8-core shard_map executor.

    Returns (run, in_names, out_names, out_shapes) where
    run(concat_inputs: list[np.ndarray]) -> list of per-output global arrays.
    """
    key = (b2_nonzero, b1_nonzero)
    if key in _CACHE:
        return _CACHE[key]
    import jax
    from jax.sharding import Mesh, PartitionSpec
    from jax.experimental.shard_map import shard_map
    from concourse import bass2jax
    from concourse.bass2jax import (_bass_exec_p, install_neuronx_cc_hook,
                                    partition_id_tensor)

    nc = _build(b2_nonzero=b2_nonzero, b1_nonzero=b1_nonzero)
    install_neuronx_cc_hook()

    part_name = nc.partition_id_tensor.name if nc.partition_id_tensor else None
    in_names, out_names, out_avals = [], [], []
    for alloc in nc.m.functions[0].allocations:
        if not isinstance(alloc, mybir.MemoryLocationSet):
            continue
        name = alloc.memorylocations[0].name
        if alloc.kind == "ExternalInput":
            if name != part_name:
                in_names.append(name)
        elif alloc.kind == "ExternalOutput":
            out_names.append(name)
            out_avals.append(jax.core.ShapedArray(
                tuple(alloc.tensor_shape), mybir.dt.np(alloc.dtype)))
    n_params = len(in_names)
    n_outs = len(out_names)
    all_in_names = in_names + out_names
    if part_name is not None:
        all_in_names = all_in_names + [part_name]

    def _body(*args):
        operands = list(args)
        if part_name is not None:
            operands.append(partition_id_tensor())
        outs = _bass_exec_p.bind(
            *operands,
            out_avals=tuple(out_avals),
            in_names=tuple(all_in_names),
            out_names=tuple(out_names),
            lowering_input_output_aliases=(),
            sim_require_finite=True,
            sim_require_nnan=True,
            nc=nc,
        )
        return tuple(outs)

    devices = jax.devices()[:N_CORES]
    mesh = Mesh(np.asarray(devices), ("core",))
    donate = tuple(range(n_params, n_params + n_outs))
    sharded = jax.jit(
        shard_map(_body, mesh=mesh,
                  in_specs=(PartitionSpec("core"),) * (n_params + n_outs),
                  out_specs=(PartitionSpec("core"),) * n_outs,
                  check_rep=False),
        donate_argnums=donate, keep_unused=True)

    def run(concat_inputs):
        zeros = [np.zeros((N_CORES * a.shape[0], *a.shape[1:]), a.dtype)
                 for a in out_avals]
        outs = sharded(*concat_inputs, *zeros)
        return [np.asarray(o) for o in outs]

    out_shapes = [tuple(a.shape) for a in out_avals]
    _CACHE[key] = (run, in_names, out_names, out_shapes)
    _CACHE[(key, "bench")] = (sharded, out_avals)
    return _CACHE[key]


def kernel(x, eps, W0, b0, W1, b1, W2, b2):
    x = np.asarray(x, np.float32)
    eps = np.asarray(eps, np.float32)
    W0 = np.asarray(W0, np.float32)
    b0 = np.asarray(b0, np.float32)
    W1 = np.asarray(W1, np.float32)
    b1 = np.asarray(b1, np.float32)
    W2 = np.asarray(W2, np.float32)
    b2 = np.asarray(b2, np.float32)

    b2_nonzero = bool(np.any(b2))
    b1_nonzero = bool(np.any(b1))
    run, in_names, out_names, _ = _get_executor(b2_nonzero, b1_nonzero)

    # host-side weight prep
    w0d = _round_f32r(np.concatenate([W0[:D], W0[:D]], axis=0))      # [128, 256]
    # L1 bias table: col j = b0 + t_j * W0[64], t_j = j*dt/2, plus b1 folded? no.
    tgrid = (np.arange(2 * NUM_STEPS + 1, dtype=np.float64) * (DT / 2))
    b0t = (b0[None, :].astype(np.float64)
           + tgrid[:, None] * W0[D].astype(np.float64)).astype(np.float32)  # [33, H]
    b0t = np.ascontiguousarray(b0t.T)                                 # [H, 33]
    # NOTE: b1 folding - L2 bias is b1 (zeros in spec); if nonzero we add via
    # the relu bias path. Handled below by asserting zero for the fast path.
    w1r = _round_f32r(W1)
    w2a = np.zeros((H, 128), np.float32)
    w2a[:, :D] = W2
    w2b = np.zeros((H, 128), np.float32)
    w2b[:, D:] = W2
    w2a = _round_f32r(w2a)
    w2b = _round_f32r(w2b)
    onesld = np.zeros((128, 2), np.float32)
    c_star = -DT * 0.5  # trace = 0.5*e.(e+Jmlp e); ld -= dt*trace
    onesld[0:64, 0] = c_star
    onesld[64:128, 1] = c_star
    onesld = _round_f32r(onesld)
    ident = np.eye(128, dtype=np.float32)
    b2d = np.concatenate([b2, b2]).reshape(128, 1).astype(np.float32)

    # Global (concatenated over cores) input arrays for shard_map:
    # per-core eps shard is eps[:, r0:r0+B_CORE, :]; concatenated along axis 0
    # that is eps transposed to [cores*steps, B_CORE, D].
    eps_g = np.ascontiguousarray(
        eps.reshape(NUM_STEPS, N_CORES, B_CORE, D).transpose(1, 0, 2, 3)
    ).reshape(N_CORES * NUM_STEPS, B_CORE, D)
    per_name = {
        "x": x,                               # [8*8192, 64] already global
        "eps": eps_g,
        "w0d": np.tile(w0d, (N_CORES, 1)),
        "b0t": np.tile(b0t, (N_CORES, 1)),
        "w1": np.tile(w1r, (N_CORES, 1)),
        "w2a": np.tile(w2a, (N_CORES, 1)),
        "w2b": np.tile(w2b, (N_CORES, 1)),
        "onesld": np.tile(onesld, (N_CORES, 1)),
        "ident": np.tile(ident, (N_CORES, 1)),
        "b2d": np.tile(b2d, (N_CORES, 1)),
        "b1d": np.tile(b1.reshape(H, 1).astype(np.float32), (N_CORES, 1)),
    }
    outs = run([per_name[n] for n in in_names])
    res = dict(zip(out_names, outs))
    x_out = res["xo"]
    log_det = res["ld"]
    return x_out, log_det
